# revision 1
# baseline (speedup 1.0000x reference)
"""Bi-directional MinGRU kernel for Trainium2 (8 NeuronCores, SPMD).

Problem: x [4, 4096, 1024]; per direction d in {fwd, bwd}:
    k  = x @ Wz_d + bz_d
    A  = sigmoid(-k)           (= 1 - z, the carry coefficient)
    z  = sigmoid(k)
    gp = x @ Wh_d + bh_d
    g  = max(gp + 0.5, sigmoid(gp))      (== where(gp>=0, gp+0.5, sigmoid(gp)))
    h_t = A_t * h_{t-1} + z_t * g_t      (linear first-order scan over S)
    out = concat(h_fwd, h_bwd) @ W_out + b_out

Sharding: 8 cores = (4 batches) x (2 directions). Each core computes the
full hidden state for one (batch, direction) and its half of the final
2H->H projection; the two partial products per batch are summed on host.

Per-core layout: everything is kept transposed ([channel, seq]) so the
sequential scan runs along the free dimension with channels on partitions,
using the native VectorE tensor_tensor_scan instruction.
"""

import os
import numpy as np
from contextlib import ExitStack

import concourse.bass as bass
import concourse.tile as tile
from concourse import bacc, mybir
from concourse.bass_utils import run_bass_kernel_spmd

P = 128          # partitions
S = 4096         # sequence length
D = 1024         # input dim
H = 1024         # hidden dim
SC = 512         # seq chunk (one PSUM bank of fp32)
NSC = S // SC    # 8 seq chunks
ND = D // P      # 8 contraction tiles for GEMM1
NH = H // P      # 8 hidden tiles
NCORES = 8

F32 = mybir.dt.float32

# matmul input modes:
#   "f32r"   - all matmul inputs float32r (fp32 bytes, 1 cyc/row PE path)
#   "hybrid" - gate GEMMs (x, Wz, Wh) in bf16 (their error is damped by the
#              sigmoids), output GEMM (h, Wo) in float32r
#   "bf16"   - everything bf16
# float32r must be declared end-to-end (walrus birverifier requires the
# producer chain to be f32r-typed); the raw bytes are plain fp32.
# Default bf16: l2-rel 2.7e-3 => resid_var 7.2e-6, 14x under the
# concourse-standard gate (resid_var < 1e-4); fastest measured config.
MM_MODE = os.environ.get("BIMINGRU_MM_MODE", "bf16")

BF16 = mybir.dt.bfloat16
F32R = mybir.dt.float32r
if MM_MODE == "bf16":
    X_DT, O_DT = BF16, BF16
elif MM_MODE == "hybrid":
    X_DT, O_DT = BF16, F32R
elif MM_MODE == "f32r":
    X_DT, O_DT = F32R, F32R
else:
    X_DT, O_DT = F32, F32
H_DT = O_DT                      # scan output dtype (GEMM3 rhs)


def _np_dt(dt):
    if dt == BF16:
        import ml_dtypes
        return np.dtype(ml_dtypes.bfloat16)
    return np.dtype(np.float32)


def _mm(ap):
    return ap


def _build_module():
    nc = bacc.Bacc("TRN2", target_bir_lowering=False, debug=False)

    # All inputs are host-blocked so every SBUF working set is ONE contiguous
    # DMA (the sync engine's ~0.65us per-DMA issue cost dominates the ramp):
    #   xT row j*128+p, col d*512+c   = x^T[d*128+p, j*512+c]   (chunk-blocked)
    #   Wz/Wh row i*128+p, col d*128+c = W[d*128+p, i*128+c]    (i-blocked)
    #   Wo row o*128+p, col i*128+c    = W_half[i*128+p, o*128+c] (o-blocked)
    #   biasT [128, 4*NH] = [bz | -bz | bh | bh+0.5] per-partition columns
    xT = nc.dram_tensor("xT", [D, S], X_DT, kind="ExternalInput").ap()
    Wz = nc.dram_tensor("Wz", [D, H], X_DT, kind="ExternalInput").ap()
    Wh = nc.dram_tensor("Wh", [D, H], X_DT, kind="ExternalInput").ap()
    Wo = nc.dram_tensor("Wo", [H, H], O_DT, kind="ExternalInput").ap()
    biasT = nc.dram_tensor("biasT", [P, 4 * NH], F32, kind="ExternalInput").ap()
    outT = nc.dram_tensor("outT", [H, S], F32, kind="ExternalOutput").ap()

    AF = mybir.ActivationFunctionType
    OP = mybir.AluOpType

    with tile.TileContext(nc) as tc, ExitStack() as ctx:
        wpool = ctx.enter_context(tc.tile_pool(name="w", bufs=1))
        xpool = ctx.enter_context(tc.tile_pool(name="x", bufs=2))
        pspool = ctx.enter_context(tc.tile_pool(name="ps", bufs=2, space="PSUM"))
        ewpool = ctx.enter_context(tc.tile_pool(name="ew", bufs=2))
        hpool = ctx.enter_context(tc.tile_pool(name="h", bufs=2))
        opool = ctx.enter_context(tc.tile_pool(name="o", bufs=3))

        # --- PE warm-up: the first real matmul can't start until ~12us of
        # input DMA lands, and a cold PE then runs at 1.2GHz for another
        # ~3.4us (HAM). Burn that idle window with dummy matmuls on
        # memset-zero tiles so the HAM un-throttles before real work
        # arrives. The dummy PSUM tile reuses the psK tag (no extra bank).
        wdum = ewpool.tile([P, P], X_DT, tag="wdum", name="wdum")
        nc.vector.memset(wdum[:], 0)
        rdum = ewpool.tile([P, SC], X_DT, tag="rdum", name="rdum")
        nc.vector.memset(rdum[:], 0)
        # 10 dummies: they pace at ~0.33us each (WAW-serialized), so this
        # ends ~11.5us — right when HAM warms (8.1+3.4) and the first real
        # operands land; more would push the real stream start back.
        psdum = pspool.tile([P, SC], F32, tag="psK", bufs=3, name="psdum")
        for _ in range(10):
            nc.tensor.matmul(psdum[:], wdum[:], rdum[:], start=True, stop=True)

        x_chunks = {}

        def load_x_chunk(j):
            # one DMA per chunk: [128, ND*SC] with free dim (d, c)
            xt = xpool.tile([P, ND * SC], X_DT, tag="xb", name=f"xb_{j}")
            nc.sync.dma_start(xt[:], xT[j * P:(j + 1) * P, :])
            x_chunks[j] = xt

        # Startup: x chunk 0 is on the critical path to the first matmul, so
        # split it into 4 DMAs (parallel queues + per-MM wait granularity);
        # then the i-blocked Wz/Wh tiles interleaved — K(0,i) unblocks as
        # soon as block WzB[i] lands, so the PE ramps with the DMA stream.
        Wz_t, Wh_t, Wo_t = [], [], []
        xt0 = xpool.tile([P, ND * SC], X_DT, tag="xb", name="xb_0")
        QS = ND * SC // 4
        nc.sync.dma_start(xt0[:, 0:QS], xT[0:P, 0:QS])
        wzt = wpool.tile([P, H], X_DT, tag="wz0", name="wz0")
        nc.sync.dma_start(wzt[:], Wz[0:P, :])
        Wz_t.append(wzt)
        for q in range(1, 4):
            nc.sync.dma_start(xt0[:, q * QS:(q + 1) * QS],
                              xT[0:P, q * QS:(q + 1) * QS])
        x_chunks[0] = xt0

        bias_sb = wpool.tile([P, 4 * NH], F32, tag="bias", name="bias_sb")
        nc.sync.dma_start(bias_sb[:], biasT[:, :])
        bz_sb = bias_sb[:, 0:NH]
        nbz_sb = bias_sb[:, NH:2 * NH]
        bh_sb = bias_sb[:, 2 * NH:3 * NH]
        bh5_sb = bias_sb[:, 3 * NH:4 * NH]

        for i in range(1, NH):
            wzt = wpool.tile([P, H], X_DT, tag=f"wz{i}", name=f"wz{i}")
            nc.sync.dma_start(wzt[:], Wz[i * P:(i + 1) * P, :])
            Wz_t.append(wzt)
            wht = wpool.tile([P, H], X_DT, tag=f"wh{i-1}", name=f"wh{i-1}")
            nc.sync.dma_start(wht[:], Wh[(i - 1) * P:i * P, :])
            Wh_t.append(wht)
        wht = wpool.tile([P, H], X_DT, tag=f"wh{NH-1}", name=f"wh{NH-1}")
        nc.sync.dma_start(wht[:], Wh[(NH - 1) * P:NH * P, :])
        Wh_t.append(wht)

        def load_wo():
            for o in range(NH):
                wot = wpool.tile([P, H], O_DT, tag=f"wo{o}", name=f"wo{o}")
                nc.sync.dma_start(wot[:], Wo[o * P:(o + 1) * P, :])
                Wo_t.append(wot)

        h_tiles = [[None] * NH for _ in range(NSC)]

        stash = {}

        def emit_k(j, i):
            xc = x_chunks[j]
            psK = pspool.tile([P, SC], F32, tag="psK", bufs=3,
                              name=f"psK_{j}_{i}")
            for d in range(ND):
                nc.tensor.matmul(
                    psK[:], _mm(Wz_t[i][:, d * P:(d + 1) * P]),
                    _mm(xc[:, d * SC:(d + 1) * SC]),
                    start=(d == 0), stop=(d == ND - 1))
            A = ewpool.tile([P, SC], F32, tag="A", bufs=3, name=f"A_{j}_{i}")
            nc.scalar.activation(A[:], psK[:], AF.Sigmoid,
                                 bias=nbz_sb[:, i:i + 1], scale=-1.0)
            z = ewpool.tile([P, SC], F32, tag="z", bufs=3, name=f"z_{j}_{i}")
            nc.scalar.activation(z[:], psK[:], AF.Sigmoid,
                                 bias=bz_sb[:, i:i + 1], scale=1.0)
            stash[(j, i)] = (A, z)

        def emit_g(j, i):
            xc = x_chunks[j]
            psG = pspool.tile([P, SC], F32, tag="psG", bufs=3,
                              name=f"psG_{j}_{i}")
            for d in range(ND):
                nc.tensor.matmul(
                    psG[:], _mm(Wh_t[i][:, d * P:(d + 1) * P]),
                    _mm(xc[:, d * SC:(d + 1) * SC]),
                    start=(d == 0), stop=(d == ND - 1))
            A, z = stash.pop((j, i))
            sg = ewpool.tile([P, SC], F32, tag="sg", name=f"sg_{j}_{i}")
            nc.scalar.activation(sg[:], psG[:], AF.Sigmoid,
                                 bias=bh_sb[:, i:i + 1], scale=1.0)
            g = ewpool.tile([P, SC], F32, tag="g", name=f"g_{j}_{i}")
            nc.vector.scalar_tensor_tensor(g[:], psG[:], bh5_sb[:, i:i + 1],
                                           sg[:], op0=OP.add, op1=OP.max)
            Bv = ewpool.tile([P, SC], F32, tag="B", name=f"B_{j}_{i}")
            nc.vector.tensor_tensor(Bv[:], z[:], g[:], op=OP.mult)

            ht = hpool.tile([P, SC], H_DT, tag=f"h{i}", name=f"h_{j}_{i}")
            init = 0.0 if j == 0 else h_tiles[j - 1][i][:, SC - 1:SC]
            nc.vector.tensor_tensor_scan(ht[:], A[:], Bv[:], initial=init,
                                         op0=OP.mult, op1=OP.add)
            h_tiles[j][i] = ht

        def emit_o(j, o):
            psO = pspool.tile([P, SC], F32, tag="psO", name=f"psO_{j}_{o}")
            for i in range(NH):
                nc.tensor.matmul(
                    psO[:], _mm(Wo_t[o][:, i * P:(i + 1) * P]),
                    _mm(h_tiles[j][i][:]),
                    start=(i == 0), stop=(i == NH - 1))
            oc = opool.tile([P, SC], F32, tag="oc", name=f"oc_{j}_{o}")
            nc.scalar.copy(oc[:], psO[:])
            nc.sync.dma_start(outT[o * P:(o + 1) * P, j * SC:(j + 1) * SC], oc[:])

        # Software pipeline. Per chunk j the PE group order is
        #   K0 K1 [G0 O0] [K2 G1 O1] [K3 G2 O2] ... [K7 G6 O6] [G7 O7]
        # where O* are the GEMM3 groups of chunk j-1. Interleaving the O
        # groups keeps ~2 PE groups between G(i) and the DVE/ACT chain that
        # releases its PSUM bank, so the PE never stalls on the elementwise
        # tail. x(j+1) is prefetched at the head of chunk j; Wo loads are
        # issued at the head of chunk 1 (first needed by GEMM3 of chunk 0).
        for j in range(NSC):
            if j + 1 < NSC:
                load_x_chunk(j + 1)
            if j == 1:
                load_wo()
            emit_k(j, 0)
            emit_k(j, 1)
            for i in range(NH):
                if i + 2 < NH:
                    emit_k(j, i + 2)
                emit_g(j, i)
                if j >= 1:
                    emit_o(j - 1, i)
        for o in range(NH - 1):
            emit_o(NSC - 1, o)
        # final O group split into two N=256 halves so the first half's
        # copy+store overlaps the second half's matmuls (shorter serial
        # tail before the drain barrier); PSUM/SBUF tags are reused so no
        # extra banks are allocated
        j, o = NSC - 1, NH - 1
        HC = SC // 2
        for half in range(2):
            psO = pspool.tile([P, HC], F32, tag="psO", name=f"psOt_{half}")
            for i in range(NH):
                nc.tensor.matmul(
                    psO[:], _mm(Wo_t[o][:, i * P:(i + 1) * P]),
                    _mm(h_tiles[j][i][:, half * HC:(half + 1) * HC]),
                    start=(i == 0), stop=(i == NH - 1))
            oc = opool.tile([P, HC], F32, tag="oc", name=f"oct_{half}")
            nc.scalar.copy(oc[:], psO[:])
            nc.sync.dma_start(
                outT[o * P:(o + 1) * P,
                     j * SC + half * HC:j * SC + (half + 1) * HC], oc[:])

    nc.compile()
    return nc


_CACHE = {}


def _get_module():
    if "nc" not in _CACHE:
        _CACHE["nc"] = _build_module()
    return _CACHE["nc"]


def _make_in_maps(x, Wz_f, bz_f, Wh_f, bh_f, Wz_b, bz_b, Wh_b, bh_b, W_out, b_out):
    np_x = _np_dt(X_DT)
    np_o = _np_dt(O_DT)
    f32 = np.float32

    def blk_w(w, dt):
        # [D, H] -> blocked: out[i*128+p, d*128+c] = w[d*128+p, i*128+c]
        w = np.asarray(w, dtype=f32).reshape(ND, P, NH, P)
        return np.ascontiguousarray(
            w.transpose(2, 1, 0, 3).reshape(H, D), dtype=dt)

    def blk_x(xb, rev):
        # [S, D] -> blocked: out[j*128+p, d*512+c] = x[j*512+c, d*128+p]
        if rev:
            xb = xb[::-1]
        xb = xb.reshape(NSC, SC, ND, P)
        return np.ascontiguousarray(
            xb.transpose(0, 3, 2, 1).reshape(NSC * P, ND * SC), dtype=np_x)

    x = np.asarray(x, dtype=f32)
    Wz_fc, Wh_fc = blk_w(Wz_f, np_x), blk_w(Wh_f, np_x)
    Wz_bc, Wh_bc = blk_w(Wz_b, np_x), blk_w(Wh_b, np_x)
    W_out = np.asarray(W_out)
    Wo_fc = blk_w(W_out[:H], np_o)      # fwd half rows of W_out
    Wo_bc = blk_w(W_out[H:], np_o)      # bwd half rows

    def bias_pack(b_z, b_h):
        def col(v):  # [H] -> [128, NH] with col i = h-tile i
            return np.asarray(v, dtype=f32).reshape(NH, P).T
        b_z = np.asarray(b_z, dtype=f32)
        b_h = np.asarray(b_h, dtype=f32)
        return {"biasT": np.ascontiguousarray(np.concatenate(
            [col(b_z), col(-b_z), col(b_h), col(b_h + 0.5)], axis=1))}

    bias_f = bias_pack(bz_f, bh_f)
    bias_b = bias_pack(bz_b, bh_b)

    in_maps = []
    for b in range(4):
        xT_f = blk_x(x[b], rev=False)
        xT_b = blk_x(x[b], rev=True)
        in_maps.append({"xT": xT_f, "Wz": Wz_fc, "Wh": Wh_fc, "Wo": Wo_fc,
                        **bias_f})
        in_maps.append({"xT": xT_b, "Wz": Wz_bc, "Wh": Wh_bc, "Wo": Wo_bc,
                        **bias_b})
    return in_maps


def _assemble(results, b_out):
    out = np.empty((4, S, H), np.float32)
    for b in range(4):
        out[b] = results[2 * b]["outT"].T
        out[b] += results[2 * b + 1]["outT"].T
    out += np.asarray(b_out, dtype=np.float32)
    return out


def kernel(x, Wz_f, bz_f, Wh_f, bh_f, Wz_b, bz_b, Wh_b, bh_b, W_out, b_out):
    nc = _get_module()
    in_maps = _make_in_maps(x, Wz_f, bz_f, Wh_f, bh_f,
                            Wz_b, bz_b, Wh_b, bh_b, W_out, b_out)
    res = run_bass_kernel_spmd(nc, in_maps, core_ids=list(range(NCORES)))
    return _assemble(res.results, b_out)



# revision 10
# speedup vs baseline: 1.2318x; 1.2318x over previous
"""Bi-directional MinGRU kernel for Trainium2 (8 NeuronCores, SPMD).

Problem: x [4, 4096, 1024]; per direction d in {fwd, bwd}:
    k  = x @ Wz_d + bz_d
    A  = sigmoid(-k)           (= 1 - z, the carry coefficient)
    z  = sigmoid(k)
    gp = x @ Wh_d + bh_d
    g  = max(gp + 0.5, sigmoid(gp))      (== where(gp>=0, gp+0.5, sigmoid(gp)))
    h_t = A_t * h_{t-1} + z_t * g_t      (linear first-order scan over S)
    out = concat(h_fwd, h_bwd) @ W_out + b_out

Sharding: 8 cores = (4 batches) x (2 directions). Each core computes the
full hidden state for one (batch, direction) and its half of the final
2H->H projection; the two partial products per batch are summed on host.

Per-core layout: everything is kept transposed ([channel, seq]) so the
sequential scan runs along the free dimension with channels on partitions,
using the native VectorE tensor_tensor_scan instruction.

Precision plan (HW-validated: fp8-e4m3 DoubleRow matmul = 2 contraction
rows/cycle = 2x bf16 throughput; numpy sim of the quantization error
matches HW to 3 digits):
  - Wz gate GEMM: full fp8 DoubleRow (error damped by sigmoid' <= 1/4).
  - Wh gate GEMM: FH of 8 contraction blocks in fp8 DoubleRow, the rest
    bf16 (the candidate path has slope-1 error passthrough, so it gets
    a precision budget).
  - Output GEMM: bf16 (h quantization error is amplified by the positive
    mean of h; fp8 fails here).
  - All fp8/bf16 gate operands carry power-of-2 scales (x*16, W*32) so
    e4m3's subnormal floor is avoided; the 1/512 un-scale folds into the
    ACT sigmoid `scale` and one DVE tensor_scalar for the linear branch.
"""

import os
import numpy as np
from contextlib import ExitStack

import concourse.bass as bass
import concourse.tile as tile
from concourse import bacc, mybir
from concourse.bass_utils import run_bass_kernel_spmd

P = 128          # partitions
S = 4096         # sequence length
D = 1024         # input dim
H = 1024         # hidden dim
SC = 512         # seq chunk (one PSUM bank of fp32)
NSC = S // SC    # 8 seq chunks
ND = D // P      # 8 contraction tiles for GEMM1
NH = H // P      # 8 hidden tiles
NCORES = 8

F32 = mybir.dt.float32
BF16 = mybir.dt.bfloat16
F8 = mybir.dt.float8e4

FH = int(os.environ.get("BIMINGRU_FH", "2"))   # Wh fp8 d-blocks (even, 0..8)
assert FH % 2 == 0 and 0 <= FH <= ND
NBH = ND - FH                                   # Wh bf16 d-blocks

SX = 16.0        # x pre-scale before quantization (|x|max*16 ~ 88 < 240)
SW = 32.0        # gate-weight pre-scale (std 1/32 -> ~1)
SINV = 1.0 / (SX * SW)


def _np_dt(dt):
    import ml_dtypes
    if dt == BF16:
        return np.dtype(ml_dtypes.bfloat16)
    if dt == F8:
        return np.dtype(ml_dtypes.float8_e4m3)
    return np.dtype(np.float32)


def _build_module():
    nc = bacc.Bacc("TRN2", target_bir_lowering=False, debug=False)

    # All inputs are host-blocked so every SBUF working set is ONE contiguous
    # DMA (the sync engine's ~0.65us per-DMA issue cost dominates the ramp):
    #   xT8 row j*128+p, col d*512+c    = 16*x[j*512+c, d*128+p]  (fp8)
    #   xT16 row j*128+p, col e*512+c   = 16*x[j*512+c, (FH+e)*128+p] (bf16)
    #   Wz8 row i*128+p, col d*128+c    = 32*Wz[d*128+p, i*128+c] (fp8)
    #   Wh8 row i*128+p, col d*128+c    = 32*Wh[d*128+p, i*128+c], d<FH
    #   Wh16 row i*128+p, col e*128+c   = 32*Wh[(FH+e)*128+p, i*128+c] (bf16)
    #   Wo row o*128+p, col i*128+c     = W_half[i*128+p, o*128+c] (bf16)
    #   biasT [128, 4*NH] = [bz | -bz | bh | bh+0.5] per-partition columns
    xT8 = nc.dram_tensor("xT8", [NSC * P, ND * SC], F8,
                         kind="ExternalInput").ap()
    Wz8 = nc.dram_tensor("Wz8", [H, ND * P], F8, kind="ExternalInput").ap()
    if FH > 0:
        Wh8 = nc.dram_tensor("Wh8", [H, FH * P], F8,
                             kind="ExternalInput").ap()
    if NBH > 0:
        xT16 = nc.dram_tensor("xT16", [NSC * P, NBH * SC], BF16,
                              kind="ExternalInput").ap()
        Wh16 = nc.dram_tensor("Wh16", [H, NBH * P], BF16,
                              kind="ExternalInput").ap()
    Wo = nc.dram_tensor("Wo", [H, H], BF16, kind="ExternalInput").ap()
    biasT = nc.dram_tensor("biasT", [P, 4 * NH], F32, kind="ExternalInput").ap()
    outT = nc.dram_tensor("outT", [H, S], BF16, kind="ExternalOutput").ap()

    AF = mybir.ActivationFunctionType
    OP = mybir.AluOpType
    DR = mybir.MatmulPerfMode.DoubleRow

    with tile.TileContext(nc) as tc, ExitStack() as ctx:
        wpool = ctx.enter_context(tc.tile_pool(name="w", bufs=1))
        xpool = ctx.enter_context(tc.tile_pool(name="x", bufs=2))
        pspool = ctx.enter_context(tc.tile_pool(name="ps", bufs=2, space="PSUM"))
        ewpool = ctx.enter_context(tc.tile_pool(name="ew", bufs=2))
        hpool = ctx.enter_context(tc.tile_pool(name="h", bufs=2))
        opool = ctx.enter_context(tc.tile_pool(name="o", bufs=3))

        # --- PE warm-up: the first real matmul can't start until the input
        # DMA lands, and a cold PE then runs at 1.2GHz for another ~3.4us
        # (HAM). Burn that idle window with dummy matmuls on memset-zero
        # tiles so the HAM un-throttles before real work arrives.
        wdum = ewpool.tile([P, P], BF16, tag="wdum", name="wdum")
        nc.vector.memset(wdum[:], 0)
        rdum = ewpool.tile([P, SC], BF16, tag="rdum", name="rdum")
        nc.vector.memset(rdum[:], 0)
        psdum = pspool.tile([P, SC], F32, tag="psK", bufs=3, name="psdum")
        for _ in range(10):
            nc.tensor.matmul(psdum[:], wdum[:], rdum[:], start=True, stop=True)

        x8_chunks = {}
        x16_chunks = {}

        def load_x_chunk(j):
            xt = xpool.tile([P, ND, SC], F8, tag="x8", name=f"x8_{j}")
            nc.sync.dma_start(xt[:], xT8[j * P:(j + 1) * P, :])
            x8_chunks[j] = xt
            if NBH > 0:
                xt16 = xpool.tile([P, NBH, SC], BF16, tag="x16",
                                  name=f"x16_{j}")
                nc.sync.dma_start(xt16[:], xT16[j * P:(j + 1) * P, :])
                x16_chunks[j] = xt16

        # Startup: x8 chunk 0 is on the critical path to the first matmul, so
        # split it into 4 DMAs (parallel queues + per-MM wait granularity);
        # then the i-blocked weight tiles interleaved — K(0,i) unblocks as
        # soon as block Wz8[i] lands, so the PE ramps with the DMA stream.
        Wz_t, Wh8_t, Wh16_t, Wo_t = [], [], [], []
        xt0 = xpool.tile([P, ND, SC], F8, tag="x8", name="x8_0")
        QS = ND * SC // 4
        NQ = ND // 4
        nc.sync.dma_start(xt0[:, 0:NQ, :], xT8[0:P, 0:QS])
        wzt = wpool.tile([P, ND, P], F8, tag="wz0", name="wz0")
        nc.sync.dma_start(wzt[:], Wz8[0:P, :])
        Wz_t.append(wzt)
        for q in range(1, 4):
            nc.sync.dma_start(xt0[:, q * NQ:(q + 1) * NQ, :],
                              xT8[0:P, q * QS:(q + 1) * QS])
        x8_chunks[0] = xt0

        bias_sb = wpool.tile([P, 4 * NH], F32, tag="bias", name="bias_sb")
        nc.sync.dma_start(bias_sb[:], biasT[:, :])
        bz_sb = bias_sb[:, 0:NH]
        nbz_sb = bias_sb[:, NH:2 * NH]
        bh_sb = bias_sb[:, 2 * NH:3 * NH]
        bh5_sb = bias_sb[:, 3 * NH:4 * NH]

        # x16 chunk 0 (needed by G(0,0) shortly after K(0,0))
        if NBH > 0:
            xt160 = xpool.tile([P, NBH, SC], BF16, tag="x16", name="x16_0")
            hq = NBH * SC // 2
            nc.sync.dma_start(xt160[:, 0:NBH // 2, :], xT16[0:P, 0:hq])
            nc.sync.dma_start(xt160[:, NBH // 2:NBH, :], xT16[0:P, hq:2 * hq])
            x16_chunks[0] = xt160

        def load_wh(i):
            if FH > 0:
                wht = wpool.tile([P, FH, P], F8, tag=f"wh8_{i}",
                                 name=f"wh8_{i}")
                nc.sync.dma_start(wht[:], Wh8[i * P:(i + 1) * P, :])
                Wh8_t.append(wht)
            if NBH > 0:
                wht16 = wpool.tile([P, NBH, P], BF16, tag=f"wh16_{i}",
                                   name=f"wh16_{i}")
                nc.sync.dma_start(wht16[:], Wh16[i * P:(i + 1) * P, :])
                Wh16_t.append(wht16)

        load_wh(0)
        for i in range(1, NH):
            wzt = wpool.tile([P, ND, P], F8, tag=f"wz{i}", name=f"wz{i}")
            nc.sync.dma_start(wzt[:], Wz8[i * P:(i + 1) * P, :])
            Wz_t.append(wzt)
            load_wh(i)

        def load_wo():
            for o in range(NH):
                wot = wpool.tile([P, H], BF16, tag=f"wo{o}", name=f"wo{o}")
                nc.sync.dma_start(wot[:], Wo[o * P:(o + 1) * P, :])
                Wo_t.append(wot)

        h_tiles = [[None] * NH for _ in range(NSC)]

        stash = {}

        def emit_k(j, i):
            xc = x8_chunks[j]
            psK = pspool.tile([P, SC], F32, tag="psK", bufs=3,
                              name=f"psK_{j}_{i}")
            for d in range(0, ND, 2):
                nc.tensor.matmul(
                    psK[:], Wz_t[i][:, d:d + 2, :], xc[:, d:d + 2, :],
                    start=(d == 0), stop=(d == ND - 2), perf_mode=DR)
            A = ewpool.tile([P, SC], F32, tag="A", bufs=3, name=f"A_{j}_{i}")
            nc.scalar.activation(A[:], psK[:], AF.Sigmoid,
                                 bias=nbz_sb[:, i:i + 1], scale=-SINV)
            z = ewpool.tile([P, SC], F32, tag="z", bufs=3, name=f"z_{j}_{i}")
            nc.scalar.activation(z[:], psK[:], AF.Sigmoid,
                                 bias=bz_sb[:, i:i + 1], scale=SINV)
            stash[(j, i)] = (A, z)

        def emit_g(j, i):
            psG = pspool.tile([P, SC], F32, tag="psG", bufs=3,
                              name=f"psG_{j}_{i}")
            nmm = FH // 2 + NBH
            m = 0
            for d in range(0, FH, 2):
                nc.tensor.matmul(
                    psG[:], Wh8_t[i][:, d:d + 2, :],
                    x8_chunks[j][:, d:d + 2, :],
                    start=(m == 0), stop=(m == nmm - 1), perf_mode=DR)
                m += 1
            for e in range(NBH):
                nc.tensor.matmul(
                    psG[:], Wh16_t[i][:, e, :], x16_chunks[j][:, e, :],
                    start=(m == 0), stop=(m == nmm - 1))
                m += 1
            A, z = stash.pop((j, i))
            sg = ewpool.tile([P, SC], F32, tag="sg", name=f"sg_{j}_{i}")
            nc.scalar.activation(sg[:], psG[:], AF.Sigmoid,
                                 bias=bh_sb[:, i:i + 1], scale=SINV)
            gc = ewpool.tile([P, SC], F32, tag="gc", name=f"gc_{j}_{i}")
            nc.vector.tensor_scalar(out=gc[:], in0=psG[:], scalar1=SINV,
                                    scalar2=bh5_sb[:, i:i + 1],
                                    op0=OP.mult, op1=OP.add)
            g = ewpool.tile([P, SC], F32, tag="g", name=f"g_{j}_{i}")
            nc.vector.tensor_tensor(g[:], gc[:], sg[:], op=OP.max)
            Bv = ewpool.tile([P, SC], F32, tag="B", name=f"B_{j}_{i}")
            nc.vector.tensor_tensor(Bv[:], z[:], g[:], op=OP.mult)

            ht = hpool.tile([P, SC], BF16, tag=f"h{i}", name=f"h_{j}_{i}")
            init = 0.0 if j == 0 else h_tiles[j - 1][i][:, SC - 1:SC]
            nc.vector.tensor_tensor_scan(ht[:], A[:], Bv[:], initial=init,
                                         op0=OP.mult, op1=OP.add)
            h_tiles[j][i] = ht

        def emit_o(j, o):
            psO = pspool.tile([P, SC], F32, tag="psO", name=f"psO_{j}_{o}")
            for i in range(NH):
                nc.tensor.matmul(
                    psO[:], Wo_t[o][:, i * P:(i + 1) * P],
                    h_tiles[j][i][:],
                    start=(i == 0), stop=(i == NH - 1))
            oc = opool.tile([P, SC], BF16, tag="oc", name=f"oc_{j}_{o}")
            nc.scalar.copy(oc[:], psO[:])
            nc.sync.dma_start(outT[o * P:(o + 1) * P, j * SC:(j + 1) * SC], oc[:])

        # Software pipeline. Per chunk j the PE group order is
        #   K0 K1 [G0 O0] [K2 G1 O1] [K3 G2 O2] ... [K7 G6 O6] [G7 O7]
        # where O* are the GEMM3 groups of chunk j-1. Interleaving the O
        # groups keeps ~2 PE groups between G(i) and the DVE/ACT chain that
        # releases its PSUM bank, so the PE never stalls on the elementwise
        # tail. x(j+1) is prefetched at the head of chunk j; Wo loads are
        # issued at the head of chunk 1 (first needed by GEMM3 of chunk 0).
        for j in range(NSC):
            if j + 1 < NSC:
                load_x_chunk(j + 1)
            if j == 1:
                load_wo()
            emit_k(j, 0)
            emit_k(j, 1)
            for i in range(NH):
                if i + 2 < NH:
                    emit_k(j, i + 2)
                emit_g(j, i)
                if j >= 1:
                    emit_o(j - 1, i)
        for o in range(NH - 1):
            emit_o(NSC - 1, o)
        # final O group split into two N=256 halves so the first half's
        # copy+store overlaps the second half's matmuls (shorter serial
        # tail before the drain barrier); PSUM/SBUF tags are reused so no
        # extra banks are allocated
        j, o = NSC - 1, NH - 1
        HC = SC // 2
        for half in range(2):
            psO = pspool.tile([P, HC], F32, tag="psO", name=f"psOt_{half}")
            for i in range(NH):
                nc.tensor.matmul(
                    psO[:], Wo_t[o][:, i * P:(i + 1) * P],
                    h_tiles[j][i][:, half * HC:(half + 1) * HC],
                    start=(i == 0), stop=(i == NH - 1))
            oc = opool.tile([P, HC], BF16, tag="oc", name=f"oct_{half}")
            nc.scalar.copy(oc[:], psO[:])
            nc.sync.dma_start(
                outT[o * P:(o + 1) * P,
                     j * SC + half * HC:j * SC + (half + 1) * HC], oc[:])

    nc.compile()
    return nc


_CACHE = {}


def _get_module():
    if "nc" not in _CACHE:
        _CACHE["nc"] = _build_module()
    return _CACHE["nc"]


def _make_in_maps(x, Wz_f, bz_f, Wh_f, bh_f, Wz_b, bz_b, Wh_b, bh_b, W_out, b_out):
    import ml_dtypes
    np_f8 = np.dtype(ml_dtypes.float8_e4m3)
    np_bf = np.dtype(ml_dtypes.bfloat16)
    f32 = np.float32

    def blk_w(w, dt, scale=1.0, dlo=0, dhi=ND):
        # [D, H] -> blocked: out[i*128+p, d*128+c] = scale*w[d*128+p, i*128+c]
        w = np.asarray(w, dtype=f32)
        if scale != 1.0:
            w = w * f32(scale)
        w = w.reshape(ND, P, NH, P)[dlo:dhi]
        return np.ascontiguousarray(
            w.transpose(2, 1, 0, 3).reshape(H, (dhi - dlo) * P), dtype=dt)

    def blk_x(xb, rev, dt, dlo=0, dhi=ND):
        # [S, D] -> blocked: out[j*128+p, d*512+c] = 16*x[j*512+c, d*128+p]
        if rev:
            xb = xb[::-1]
        xb = (xb * f32(SX)).reshape(NSC, SC, ND, P)
        xb = xb.transpose(0, 3, 2, 1)[:, :, dlo:dhi, :]
        return np.ascontiguousarray(
            xb.reshape(NSC * P, (dhi - dlo) * SC), dtype=dt)

    x = np.asarray(x, dtype=f32)
    W_out = np.asarray(W_out)

    def gate_maps(Wz, Wh):
        m = {"Wz8": blk_w(Wz, np_f8, SW)}
        if FH > 0:
            m["Wh8"] = blk_w(Wh, np_f8, SW, 0, FH)
        if NBH > 0:
            m["Wh16"] = blk_w(Wh, np_bf, SW, FH, ND)
        return m

    g_f = gate_maps(Wz_f, Wh_f)
    g_b = gate_maps(Wz_b, Wh_b)
    Wo_fc = blk_w(W_out[:H], np_bf)      # fwd half rows of W_out
    Wo_bc = blk_w(W_out[H:], np_bf)      # bwd half rows

    def bias_pack(b_z, b_h):
        def col(v):  # [H] -> [128, NH] with col i = h-tile i
            return np.asarray(v, dtype=f32).reshape(NH, P).T
        b_z = np.asarray(b_z, dtype=f32)
        b_h = np.asarray(b_h, dtype=f32)
        return {"biasT": np.ascontiguousarray(np.concatenate(
            [col(b_z), col(-b_z), col(b_h), col(b_h + 0.5)], axis=1))}

    bias_f = bias_pack(bz_f, bh_f)
    bias_b = bias_pack(bz_b, bh_b)

    in_maps = []
    for b in range(4):
        for rev, g, bias, Wo_c in ((False, g_f, bias_f, Wo_fc),
                                   (True, g_b, bias_b, Wo_bc)):
            m = {"xT8": blk_x(x[b], rev, np_f8), "Wo": Wo_c, **g, **bias}
            if NBH > 0:
                m["xT16"] = blk_x(x[b], rev, np_bf, FH, ND)
            in_maps.append(m)
    return in_maps


def _assemble(results, b_out):
    out = np.empty((4, S, H), np.float32)
    for b in range(4):
        out[b] = results[2 * b]["outT"].T.astype(np.float32)
        out[b] += results[2 * b + 1]["outT"].T.astype(np.float32)
    out += np.asarray(b_out, dtype=np.float32)
    return out


def kernel(x, Wz_f, bz_f, Wh_f, bh_f, Wz_b, bz_b, Wh_b, bh_b, W_out, b_out):
    nc = _get_module()
    in_maps = _make_in_maps(x, Wz_f, bz_f, Wh_f, bh_f,
                            Wz_b, bz_b, Wh_b, bh_b, W_out, b_out)
    res = run_bass_kernel_spmd(nc, in_maps, core_ids=list(range(NCORES)))
    return _assemble(res.results, b_out)
